# revision 13
# baseline (speedup 1.0000x reference)
"""Trainium2 Bass kernel for BaseAttention (Bahdanau-style additive attention).

Reference computation (per batch row b):
    att_h  = h @ W.T + b_h                         # [B, A]
    dot    = tanh(iaf + att_h[:, None, :])         # [B, L, A]
    scores = dot @ alpha + alpha_b                 # [B, L]
    w      = softmax(scores, axis=1)               # [B, L]
    out    = sum_l w[b, l] * af[b, l, :]           # [B, D]

Sharding: data-parallel over batch, B=128 -> 16 per core across 8 cores.

The kernel is HBM-bandwidth bound; the big streamed tensors (af, iaf, W) are
downcast to fp16 on the host, halving DMA bytes (rel tolerance is 2e-2; fp16
keeps us ~3e-4).  Per-core layout (rows = (b, l) flattened, R=3136):

  - iaf (3.2MB) is loaded fully up front, so the whole scores pipeline
    (broadcast-matmul -> add -> tanh -> fused mul+reduce -> exp -> masked
    e-columns) runs ahead of the af stream, decoupled from it.  e-columns for
    all 25 row tiles persist in SBUF (100KB) until the af-paced loop consumes
    them; the af loop then only carries the 4 N=512 matmuls per tile, keeping
    the tensor engine below the DMA cadence.
  - scores via one fused DVE scalar_tensor_tensor: (tanh*1)*alpha with
    accum_out giving the row sums directly (saves a full DVE reduce pass).
  - softmax denominator deferred: e = exp(scores) unnormalized; the final
    result is (sum_l e*af) * 1/(sum_l e), with sum_l e computed as
    e_cols.T @ [1,0] in the scores loop.
  - small constants load via the scalar/vector/gpsimd DGE queues so the sync
    ring starts issuing the big streams immediately.
"""

from contextlib import ExitStack

import numpy as np

import concourse.bass as bass
import concourse.mybir as mybir
import concourse.tile as tile
from concourse import bacc
from concourse.bass_utils import run_bass_kernel_spmd

F32 = mybir.dt.float32
F16 = mybir.dt.float16
AF_T = mybir.ActivationFunctionType
ALU = mybir.AluOpType

B, L, D, A = 128, 196, 2048, 512
NCORES = 8
BPC = B // NCORES          # 16 batch rows per core
R = BPC * L                # 3136 (b, l) rows per core
P = 128                    # partitions
NT = (R + P - 1) // P      # 25 row tiles (24 full + one 64-row tail)
KCH = D // P               # 16 k-chunks for the h @ W.T matmul
DCH = 4                    # d chunks of 512 for the weighted sum
DC = D // DCH              # 512
AFG = 4                    # row tiles per streamed DMA group
TAILR = R - (NT - 1) * P   # 64 rows in the last tile


def _build_program():
    nc = bacc.Bacc(None, target_bir_lowering=False)

    h_t = nc.declare_dram_parameter("h_t", [D, BPC], F16, isOutput=False)
    w_t = nc.declare_dram_parameter("w_t", [D, A], F16, isOutput=False)
    b_bc = nc.declare_dram_parameter("b_bc", [BPC, A], F32, isOutput=False)
    alpha_bc = nc.declare_dram_parameter("alpha_bc", [P, A], F16, isOutput=False)
    alphab_bc = nc.declare_dram_parameter("alphab_bc", [P, 1], F32, isOutput=False)
    ind = nc.declare_dram_parameter("ind", [NT * P, BPC], F16, isOutput=False)
    ind_t = nc.declare_dram_parameter("ind_t", [BPC, R], F16, isOutput=False)
    iaf = nc.declare_dram_parameter("iaf", [R, A], F16, isOutput=False)
    af = nc.declare_dram_parameter("af", [R, D], F16, isOutput=False)
    out = nc.declare_dram_parameter("out", [BPC, D], F32, isOutput=True)

    with ExitStack() as ctx:
        tc = ctx.enter_context(tile.TileContext(nc))
        consts = ctx.enter_context(tc.tile_pool(name="consts", bufs=1))
        wpool = ctx.enter_context(tc.tile_pool(name="wpool", bufs=1))
        iafp = ctx.enter_context(tc.tile_pool(name="iafp", bufs=1))
        afp = ctx.enter_context(tc.tile_pool(name="afp", bufs=3))
        scr = ctx.enter_context(tc.tile_pool(name="scr", bufs=2))
        ps_bc = ctx.enter_context(
            tc.tile_pool(name="ps_bc", bufs=2, space=bass.MemorySpace.PSUM)
        )
        ps_hb = ctx.enter_context(
            tc.tile_pool(name="ps_hb", bufs=1, space=bass.MemorySpace.PSUM)
        )
        ps_acc = ctx.enter_context(
            tc.tile_pool(name="ps_acc", bufs=1, space=bass.MemorySpace.PSUM)
        )

        # --- big streams on the sync ring: W, iaf (fully resident), then af ---
        w_sb = wpool.tile([P, KCH, A], F16)
        nc.sync.dma_start(w_sb[:], w_t[:, :].rearrange("(k p) a -> p k a", p=P))

        iaf_all = iafp.tile([P, NT, A], F16)
        NFULL_T = NT - 1  # 24 full tiles
        for c in range(0, NFULL_T, AFG):
            n = min(AFG, NFULL_T - c)
            nc.sync.dma_start(
                iaf_all[:, c : c + n, :],
                iaf[c * P : (c + n) * P, :].rearrange("(t p) a -> p t a", p=P),
            )
        nc.sync.dma_start(iaf_all[:TAILR, NFULL_T, :], iaf[NFULL_T * P :, :])

        # --- small constants on the other DGE queues ---
        ht_sb = consts.tile([P, KCH, BPC], F16)
        nc.scalar.dma_start(ht_sb[:], h_t[:, :].rearrange("(k p) b -> p k b", p=P))
        bbc_sb = consts.tile([BPC, A], F32)
        nc.scalar.dma_start(bbc_sb[:], b_bc[:, :])
        abc_sb = consts.tile([P, A], F16)
        nc.gpsimd.dma_start(abc_sb[:], alpha_bc[:, :])
        abb_sb = consts.tile([P, 1], F32)
        nc.gpsimd.dma_start(abb_sb[:], alphab_bc[:, :])
        ind_sb = consts.tile([P, NT, BPC], F16)
        nc.gpsimd.dma_start(ind_sb[:], ind[:, :].rearrange("(t p) b -> p t b", p=P))
        indt_sb = consts.tile([BPC, R], F16)
        nc.gpsimd.dma_start(indt_sb[:], ind_t[:, :])

        ones2 = consts.tile([P, 2], F16)
        nc.gpsimd.memset(ones2[:, 0:1], 1.0)
        nc.gpsimd.memset(ones2[:, 1:2], 0.0)

        scores_all = consts.tile([P, NT], F32)
        e_all = consts.tile([P, NT], F32)
        ecols_all = consts.tile([P, NT, BPC], F16)

        # --- att_hb = h @ W.T + b_h, shape [BPC, A] ---
        atthb_ps = ps_hb.tile([BPC, A], F32)
        for k in range(KCH):
            nc.tensor.matmul(
                atthb_ps[:],
                ht_sb[:, k, :],
                w_sb[:, k, :],
                start=(k == 0),
                stop=(k == KCH - 1),
            )
        atthb_sb = consts.tile([BPC, A], F16)
        nc.vector.tensor_add(atthb_sb[:], atthb_ps[:], bbc_sb[:])

        # --- accumulators for the weighted sum and softmax denominator ---
        acc_ps = ps_acc.tile([BPC, DCH, DC], F32)
        sums_ps = ps_acc.tile([BPC, 2], F32)

        # --- scores pipeline: paced by iaf only ---
        for t in range(NT):
            pt = P if t < NT - 1 else TAILR
            rt = t * P

            # att_hb broadcast to this tile's rows: ind_t[:, rows].T @ att_hb
            bc_ps = ps_bc.tile([P, A], F32, tag="bc")
            nc.tensor.matmul(
                bc_ps[:pt, :],
                indt_sb[:, rt : rt + pt],
                atthb_sb[:],
                start=True,
                stop=True,
            )

            tadd = scr.tile([P, A], F16, tag="tadd")
            nc.vector.tensor_add(tadd[:pt, :], iaf_all[:pt, t, :], bc_ps[:pt, :])

            tanh = scr.tile([P, A], F16, tag="tanh")
            nc.scalar.activation(tanh[:pt, :], tadd[:pt, :], AF_T.Tanh)

            # scores[:, t] = sum_a tanh * alpha (fused mul+reduce)
            junk = scr.tile([P, A], F16, tag="junk")
            nc.vector.scalar_tensor_tensor(
                junk[:pt, :],
                tanh[:pt, :],
                1.0,
                abc_sb[:pt, :],
                op0=ALU.mult,
                op1=ALU.mult,
                accum_out=scores_all[:pt, t : t + 1],
            )

            # alpha_b folded into the Exp bias
            nc.scalar.activation(
                e_all[:pt, t : t + 1],
                scores_all[:pt, t : t + 1],
                AF_T.Exp,
                bias=abb_sb[:pt, :],
            )

            # masked weight columns: e_cols[:, b] = e * (row belongs to b)
            nc.vector.tensor_scalar_mul(
                ecols_all[:pt, t, :], ind_sb[:pt, t, :], e_all[:pt, t : t + 1]
            )

            # denominator: e_cols.T @ [1, 0]
            nc.tensor.matmul(
                sums_ps[:],
                ecols_all[:pt, t, :],
                ones2[:pt, :],
                start=(t == 0),
                stop=(t == NT - 1),
            )

        # denominator reciprocal is ready long before the af stream drains
        recip = consts.tile([BPC, 1], F32)
        nc.vector.reciprocal(recip[:], sums_ps[:, 0:1])

        # --- af-paced weighted-sum loop ---
        af_tiles = {}
        for t in range(NT):
            pt = P if t < NT - 1 else TAILR
            rt = t * P

            if t % AFG == 0:
                n = min(AFG, NT - t)
                nfull = n if t + n < NT else n - 1
                g = afp.tile([P, AFG, D], F16, tag="af")
                if nfull:
                    nc.sync.dma_start(
                        g[:, :nfull, :],
                        af[rt : rt + nfull * P, :].rearrange("(t p) d -> p t d", p=P),
                    )
                if nfull < n:
                    nc.sync.dma_start(
                        g[:TAILR, nfull, :],
                        af[rt + nfull * P : rt + nfull * P + TAILR, :],
                    )
                for jj in range(n):
                    af_tiles[t + jj] = (g, jj)

            af_g, af_j = af_tiles.pop(t)
            for c in range(DCH):
                nc.tensor.matmul(
                    acc_ps[:, c, :],
                    ecols_all[:pt, t, :],
                    af_g[:pt, af_j, c * DC : (c + 1) * DC],
                    start=(t == 0),
                    stop=(t == NT - 1),
                )

        # --- normalize (split scalar/vector) and store ---
        out_sb = consts.tile([BPC, D], F32)
        for c in range(DCH):
            dst = out_sb[:, c * DC : (c + 1) * DC]
            if c < 2:
                nc.scalar.mul(dst, acc_ps[:, c, :], recip[:])
            else:
                nc.vector.tensor_scalar_mul(dst, acc_ps[:, c, :], recip[:])
        nc.sync.dma_start(out[:, :], out_sb[:])

    nc.compile()
    return nc


_PROGRAM = None


def _get_program():
    global _PROGRAM
    if _PROGRAM is None:
        _PROGRAM = _build_program()
    return _PROGRAM


def _host_prep(h, att_feats, internal_att_feats, h2att_w, h2att_b, alpha_w, alpha_b):
    h16 = np.asarray(h, np.float32).astype(np.float16)
    af16 = np.asarray(att_feats, np.float32).astype(np.float16)
    iaf16 = np.asarray(internal_att_feats, np.float32).astype(np.float16)
    h2att_w = np.asarray(h2att_w, np.float32)
    h2att_b = np.asarray(h2att_b, np.float32)
    alpha_w = np.asarray(alpha_w, np.float32)
    alpha_b = np.asarray(alpha_b, np.float32)

    w_t = np.ascontiguousarray(h2att_w.T.astype(np.float16))   # [D, A]
    b_bc = np.tile(h2att_b.reshape(1, A), (BPC, 1)).astype(np.float32)
    alpha_bc = np.tile(alpha_w.reshape(1, A), (P, 1)).astype(np.float16)
    alphab_bc = np.full((P, 1), float(alpha_b.reshape(-1)[0]), np.float32)

    ind = np.zeros((NT * P, BPC), np.float16)
    rows = np.arange(R)
    ind[rows, rows // L] = 1.0
    ind_t = np.ascontiguousarray(ind[:R].T)                    # [BPC, R]

    in_maps = []
    for i in range(NCORES):
        sl = slice(i * BPC, (i + 1) * BPC)
        in_maps.append(
            {
                "h_t": np.ascontiguousarray(h16[sl].T),
                "w_t": w_t,
                "b_bc": b_bc,
                "alpha_bc": alpha_bc,
                "alphab_bc": alphab_bc,
                "ind": ind,
                "ind_t": ind_t,
                "iaf": iaf16[sl].reshape(R, A),
                "af": af16[sl].reshape(R, D),
            }
        )
    return in_maps


def run(trace=False, **inputs):
    """Run the SPMD kernel; returns (full_output [B, D], BassKernelResults)."""
    nc = _get_program()
    in_maps = _host_prep(**inputs)
    res = run_bass_kernel_spmd(nc, in_maps, list(range(NCORES)), trace=trace)
    out = np.concatenate([res.results[i]["out"] for i in range(NCORES)], axis=0)
    return out, res


def kernel(**inputs):
    out, _ = run(trace=False, **inputs)
    return out
